# revision 70
# baseline (speedup 1.0000x reference)
"""MultiHeadAttention Trainium2 kernel (8 NeuronCores).

Problem: b=2, n=2048, dim=1024, heads=16, dim_head=64, causal attention,
padding mask (all-ones in this problem), fp32 I/O.

Sharding (per core c in 0..7): batch b = c//4, head-group g = c%4 (4 heads).
  - attention is fully local per (batch, head-group)
  - attnout^T (bf16) is AllGathered inside each 4-core batch group, split
    into four 512-query chunks so the collectives overlap attention compute
  - each core then computes a disjoint 256-column slice of the output
    projection (Wo column split), so host reassembly is pure concatenation.

v3 structure (PE-bound early, ScalarE-exp-bound late; both phases packed):
  - host casts x and weights to bf16; inputs land via batched DMAs (one
    dma_start costs ~630ns of queue-issue time) split over sync+scalar;
    ~10 dummy matmuls on a memset tile run during the initial DMA wait so
    the HAM clock gate flips to 8/8 before the real projections start
  - Q/K projections are front-loaded (they gate the exp stream that paces
    the whole second half); V projections trail inside their own block's
    loop where the PE has slack; mb3's Q/K projections are split between
    mb1 and mb2 to balance PE load
  - the next block's first chunks' QK+exp are PRE-issued inside/at the end
    of the current block's loop so ScalarE never starves across block
    transitions (pt pool holds up to 22 exp tiles); the pre-issue is split
    per head-pair: hp0's exps are emitted right after the fi=0 projections,
    before the fi=1 projections even run
  - AV accumulates into ONE packed PSUM tile [65, 4heads, 512] whose 65th
    row is the softmax row-sum (ones-column of V)
  - mid-block normalize: Ln+Exp reciprocal on ScalarE + gpsimd broadcast
    (off critical path); FINAL block normalize is latency-optimized:
    Ln reads the rowsum row straight from PSUM (parallel with the DVE
    copy), reciprocal in bf16, partition-broadcast via a PE ones-outer-
    product into spare S-pool PSUM, ag_in store on the gpsimd queue
    (right before the collective trigger, dodging the sync-queue clog)
  - attn output stored to DRAM in (d h) row order (contiguous 4KB per
    partition); the host permutes Wo's rows to match
  - final agb loads batched over gpsimd+sync+scalar queues; out-proj of
    blocks 0..2 held back as real PE filler under the final AllGather,
    plus a short ScalarE-paced keep-warm chain (PE re-throttles to
    1.2GHz after >3.4us idle)
  - softmax runs without max subtraction: logits are ~N(0,1), exp safe
  - all matmuls bf16; S^T computed transposed so exp(S^T) feeds AV directly
"""

import numpy as np

B = 2
N = 2048
DIM = 1024
HEADS = 16
DIM_HEAD = 64
SCALE = DIM_HEAD**-0.5  # 0.125
NCORES = 8
GROUPS = 4  # head groups (cores per batch)
GDIM = DIM // GROUPS  # 256 features per core
P = 128
QB = 512  # query macroblock
NB = N // QB  # 4 q-macroblocks
KO = DIM // P  # 8 contraction chunks
JT = N // P  # 16 key tiles

_cached = None


def _build_nc():
    import concourse.mybir as mybir
    import concourse.tile as tile
    from concourse import bacc

    f32 = mybir.dt.float32
    bf16 = mybir.dt.bfloat16
    Exp = mybir.ActivationFunctionType.Exp

    nc = bacc.Bacc(num_devices=NCORES)

    # We use both Exp (attention softmax) and Ln (reciprocal via exp(-ln s)).
    # Steer the greedy table-set picker to the combined set so it never
    # thrashes (~2.7us per ACT_TABLE_LOAD otherwise).
    from concourse import hw_specs

    tables = hw_specs.get_activation_tables(nc.m.arch)
    keep = "natural_log_exp_and_others"
    Exp_f = mybir.ActivationFunctionType.Exp
    Ln_f = mybir.ActivationFunctionType.Ln
    for name, fns in tables.items():
        if name != keep:
            fns.discard(Exp_f)
            fns.discard(Ln_f)

    xT = nc.dram_tensor("xT", [DIM, N], bf16, kind="ExternalInput")
    wq = nc.dram_tensor("wq", [DIM, GDIM], bf16, kind="ExternalInput")
    wk = nc.dram_tensor("wk", [DIM, GDIM], bf16, kind="ExternalInput")
    wv = nc.dram_tensor("wv", [DIM, GDIM], bf16, kind="ExternalInput")
    wo = nc.dram_tensor("wo", [DIM, GDIM], bf16, kind="ExternalInput")
    m0 = nc.dram_tensor("m0", [P, QB], bf16, kind="ExternalInput")
    # bf16 output: halves the store traffic and the tail's final store;
    # the host casts back to f32 (adds ~0.2% RMS rounding, budget is 2%)
    outT = nc.dram_tensor("outT", [GDIM, N], bf16, kind="ExternalOutput")

    with tile.TileContext(nc) as tc:
        with (
            tc.tile_pool(name="wpool", bufs=1) as wpool,    # weights + consts
            tc.tile_pool(name="xpool", bufs=1) as xpool,    # x chunks
            tc.tile_pool(name="qkpool", bufs=1) as qkpool,  # QT/KT/V
            tc.tile_pool(name="ptpool", bufs=22) as ptpool,  # exp(S^T)
            tc.tile_pool(name="work", bufs=2) as work,      # norm staging
            tc.tile_pool(name="io", bufs=16) as io,         # agb chunks
            tc.tile_pool(name="psS", bufs=2, space="PSUM") as psS,  # 2x2 banks
            tc.tile_pool(name="psO", bufs=1, space="PSUM") as psO,  # 4 banks
            tc.tile_pool(name="dram", bufs=1, space="DRAM") as dram,
        ):
            # ---- input DMAs (bf16 direct). Each dma_start costs ~630ns of
            # queue-issue time, so batch aggressively: ni=0 x chunks stay
            # per-(k) for fine-grained gating of the first projections; the
            # rest of x goes as one [P, KO, QB] instruction per ni. Weights
            # ride the Scalar queue so issue runs in parallel with Sync. ----
            wq_bf = wpool.tile([P, KO, GDIM], bf16, name="wq_bf")
            wq_r = wq.rearrange("(ko p) f -> p ko f", p=P)
            # fi=0 half first (the first projection group needs only these
            # cols); fi=1 half right after the ni=0 x chunks
            nc.sync.dma_start(wq_bf[:, :, 0:P], wq_r[:, :, 0:P])
            xc = [[None] * NB for _ in range(KO)]
            for k in range(KO):
                t = xpool.tile([P, QB], bf16, name=f"xc{k}_0")
                nc.sync.dma_start(t[:], xT[k * P : (k + 1) * P, 0:QB])
                xc[k][0] = t
            nc.sync.dma_start(wq_bf[:, :, P:GDIM], wq_r[:, :, P:GDIM])
            # only wk competes with the critical wq+xc0 transfers — wv/wo
            # are needed much later and their early transfers starve the
            # projection stream (PE stalls re-cool the HAM clock gate)
            wk_bf = wpool.tile([P, KO, GDIM], bf16, name="wk_bf")
            nc.scalar.dma_start(wk_bf[:], wk.rearrange("(ko p) f -> p ko f", p=P))
            M0 = wpool.tile([P, QB], bf16, name="M0")
            nc.sync.dma_start(M0[:], m0[:])
            wv_bf = wpool.tile([P, KO, GDIM], bf16, name="wv_bf")
            nc.sync.dma_start(wv_bf[:], wv.rearrange("(ko p) f -> p ko f", p=P))
            xT_r = xT.rearrange("(ko p) n -> p ko n", p=P)
            xb1 = xpool.tile([P, KO, QB], bf16, name="xb1")
            nc.sync.dma_start(xb1[:], xT_r[:, :, QB : 2 * QB])
            wo_bf = wpool.tile([P, KO, GDIM], bf16, name="wo_bf")
            nc.sync.dma_start(wo_bf[:], wo.rearrange("(ko p) f -> p ko f", p=P))
            for ni in range(2, NB):
                xb = xpool.tile([P, KO, QB], bf16, name=f"xb{ni}")
                nc.sync.dma_start(xb[:], xT_r[:, :, ni * QB : (ni + 1) * QB])
                for k in range(KO):
                    xc[k][ni] = xb[:, k, :]
            for k in range(KO):
                xc[k][1] = xb1[:, k, :]

            QT = qkpool.tile([P, 2, N], bf16)
            KT = qkpool.tile([P, 2, N], bf16)
            V_sb = qkpool.tile([P, JT, GROUPS, DIM_HEAD + 1], bf16)
            nc.vector.memset(V_sb[:, :, :, DIM_HEAD : DIM_HEAD + 1], 1.0)
            ones1 = wpool.tile([1, DIM_HEAD], bf16, name="ones1")
            nc.vector.memset(ones1[:], 1.0)

            # ~4us of dummy matmuls while the first input DMAs land: flips
            # the HAM clock gate to 8/8 so the real projections start at
            # 2.4GHz instead of paying the cold 1.2GHz ramp
            warm = wpool.tile([P, QB], bf16, name="warm")
            nc.vector.memset(warm[:], 0.5)
            for i in range(14):
                wps = psS.tile([P, 1024], f32, tag="S", name="wps")[:, :QB]
                nc.tensor.matmul(wps, warm[:, 0:P], warm[:])

            # ---- deferred-emission work items ----
            def qk_group(ni, fi, which):
                def emit():
                    pq = psS.tile([P, 1024], f32, tag="S", name="pq")[:, :QB]
                    w_bf = wq_bf if which == "q" else wk_bf
                    for k in range(KO):
                        nc.tensor.matmul(
                            pq,
                            w_bf[:, k, fi * P : (fi + 1) * P],
                            xc[k][ni][:],
                            start=(k == 0),
                            stop=(k == KO - 1),
                        )
                    nsl = slice(ni * QB, (ni + 1) * QB)
                    if which == "q":
                        # fold the softmax scale into Q
                        nc.vector.tensor_scalar_mul(QT[:, fi, nsl], pq, SCALE)
                    else:
                        nc.vector.tensor_copy(KT[:, fi, nsl], pq)

                return emit

            def v_group(jt):
                def emit():
                    pv = psS.tile([P, 1024], f32, tag="S", name="pv")[:, :GDIM]
                    ni, off = divmod(jt, 4)
                    for k in range(KO):
                        nc.tensor.matmul(
                            pv,
                            xc[k][ni][:, off * P : (off + 1) * P],
                            wv_bf[:, k, :],
                            start=(k == 0),
                            stop=(k == KO - 1),
                        )
                    nc.vector.tensor_copy(
                        V_sb[:, jt, :, 0:DIM_HEAD],
                        pv.rearrange("p (h d) -> p h d", h=GROUPS),
                    )

                return emit

            ag_outs = {}

            def norm_ag(mb, po4, defer_cc=False, fast=False):
                def emit():
                    po_sb = work.tile([DIM_HEAD + 1, GROUPS, QB], f32, tag="posb")
                    lntmp = work.tile([1, GROUPS, QB], f32, tag="lntmp", bufs=1)
                    if fast:
                        # latency-critical (final) norm: Ln reads the rowsum
                        # row straight from PSUM, in parallel with the DVE
                        # copy of the 64 data rows
                        nc.scalar.activation(
                            lntmp[:],
                            po4[DIM_HEAD : DIM_HEAD + 1, :, :],
                            mybir.ActivationFunctionType.Ln,
                        )
                        nc.vector.tensor_copy(
                            po_sb[0:DIM_HEAD, :, :], po4[0:DIM_HEAD, :, :]
                        )
                    else:
                        # free the packed PSUM accumulator quickly
                        nc.vector.tensor_copy(po_sb[:], po4[:])
                    attnT = work.tile([DIM_HEAD, GROUPS, QB], bf16, tag="attnT", bufs=1)
                    if fast:
                        # reciprocal in bf16 so it can feed a PE broadcast
                        recip_bf = work.tile([1, GROUPS, QB], bf16, tag="recipb", bufs=1)
                        nc.scalar.activation(
                            recip_bf[:],
                            lntmp[:],
                            mybir.ActivationFunctionType.Exp,
                            scale=-1.0,
                        )
                        # partition-broadcast via PE (ones[1,64] outer product)
                        # ~1us and keeps the PE warm, vs 3.2us on gpsimd.
                        # lands in two spare S-pool slots (their last readers,
                        # the final chunks' exps, are long done) so there is
                        # no WAR through po4 or the po_sb copy
                        for hp in range(2):
                            bct = psS.tile([P, 1024], f32, tag="S", name=f"bc{hp}")
                            for s in range(2):
                                nc.tensor.matmul(
                                    bct[0:DIM_HEAD, s * QB : (s + 1) * QB],
                                    ones1[0:1, :],
                                    recip_bf[0:1, 2 * hp + s, :],
                                )
                            bc3 = bct[0:DIM_HEAD, :].rearrange(
                                "p (s q) -> p s q", s=2
                            )
                            nc.vector.tensor_mul(
                                attnT[:, 2 * hp : 2 * hp + 2, :],
                                po_sb[0:DIM_HEAD, 2 * hp : 2 * hp + 2, :],
                                bc3,
                            )
                    else:
                        # (DVE reciprocal alternatives measured: the native
                        # InstReciprocal takes 12.9us for [1,2048] and
                        # reciprocal_approx_fast corrupts — ScalarE Ln+Exp
                        # at ~3.8us is the best available here)
                        recip = work.tile([1, GROUPS, QB], f32, tag="recip", bufs=1)
                        nc.scalar.activation(
                            lntmp[:],
                            po_sb[DIM_HEAD : DIM_HEAD + 1, :, :],
                            mybir.ActivationFunctionType.Ln,
                        )
                        nc.scalar.activation(
                            recip[:],
                            lntmp[:],
                            mybir.ActivationFunctionType.Exp,
                            scale=-1.0,
                        )
                        bc = work.tile([DIM_HEAD, GROUPS, QB], f32, tag="bc", bufs=1)
                        # one wide broadcast (free size 2048) instead of 4 —
                        # gpsimd queue ops cost 1-4us each, so fewer is faster
                        nc.gpsimd.partition_broadcast(bc[:], recip[:])
                        nc.vector.tensor_mul(attnT[:], po_sb[0:DIM_HEAD, :, :], bc[:])
                    ag_in = dram.tile([GDIM, QB], bf16, name=f"ag_in{mb}")
                    ag_out = dram.tile([DIM, QB], bf16, name=f"ag_out{mb}")
                    # (d h) row order: partition d writes rows 4d..4d+3 as one
                    # contiguous 4KB block (the (h d) order scatters 4x 1KB).
                    # host permutes wo rows to match the gathered layout.
                    # final norm: split across gpsimd+sync so the two halves
                    # transfer in parallel (~3.4us single-DMA latency sits
                    # right before the collective trigger); the trigger's
                    # data dep waits on both writers
                    ag_r = ag_in.rearrange("(p h) q -> p h q", p=DIM_HEAD)
                    if fast:
                        nc.gpsimd.dma_start(ag_r[:, 0:2, :], attnT[:, 0:2, :])
                        nc.sync.dma_start(ag_r[:, 2:4, :], attnT[:, 2:4, :])
                    else:
                        nc.sync.dma_start(ag_r[:], attnT[:])

                    def trigger():
                        nc.gpsimd.collective_compute(
                            "AllGather",
                            mybir.AluOpType.bypass,
                            replica_groups=[[0, 1, 2, 3], [4, 5, 6, 7]],
                            ins=[ag_in.opt()],
                            outs=[ag_out.opt()],
                        )
                        ag_outs[mb] = ag_out

                    if defer_cc:
                        return trigger
                    trigger()

                return emit

            agbs = {}

            def agb_load(mb, queues=None):
                def emit():
                    # batched sub-DMAs (one dma_start costs ~630ns of queue-
                    # issue time; 8 separate loads serialized ~5us of it on
                    # the critical tail). With multiple queues (the final
                    # block), the first sub-DMA carries only chunk 0 so the
                    # out-projection's first matmul starts ~1us sooner.
                    qs = queues if queues is not None else [nc.sync]
                    t = io.tile([P, KO, QB], bf16, tag="agb", bufs=3, name=f"agb{mb}")
                    src = ag_outs[mb].rearrange("(ko p) q -> p ko q", p=P)
                    if len(qs) > 1:
                        splits = [(0, 1), (1, 3), (3, 5), (5, 8)]
                    else:
                        splits = [(0, 2), (2, 4), (4, 6), (6, 8)]
                    for i, (a, b) in enumerate(splits):
                        q = qs[i % len(qs)]
                        q.dma_start(t[:, a:b, :], src[:, a:b, :])
                    agbs[mb] = t

                return emit

            def outproj(mb, fi):
                def emit():
                    pw = psS.tile([P, 1024], f32, tag="S", name="pw")[:, :QB]
                    for k in range(KO):
                        nc.tensor.matmul(
                            pw,
                            wo_bf[:, k, fi * P : (fi + 1) * P],
                            agbs[mb][:, k, :],
                            start=(k == 0),
                            stop=(k == KO - 1),
                        )
                    ot = work.tile([P, QB], bf16, tag="ot")
                    nc.vector.tensor_copy(ot[:], pw)
                    nc.sync.dma_start(
                        outT[fi * P : (fi + 1) * P, mb * QB : (mb + 1) * QB], ot[:]
                    )

                return emit

            # (prologue moved below the qk_exp defs — it phases mb0's own
            # chunk QK+exps per head-pair just like the cross-block path)

            # one chunk's QK + exp (+causal mask) for ONE head-pair hp —
            # hp only needs the fi=hp projections, so cross-block pre-issue
            # can emit hp0 exps before the fi=1 projections even exist
            def qk_exp_hp(mb, jc, hp):
                jsl = slice(jc * P, (jc + 1) * P)
                t = jc - 4 * mb  # >= 0 on the diagonal 512-block
                cq = max(0, t) * P
                ps = psS.tile([P, 1024], f32, tag="S", name=f"ps{hp}")
                for s in range(2):
                    prow = slice(64 * s, 64 * s + 64)
                    nc.tensor.matmul(
                        ps[:, s * QB + cq : (s + 1) * QB],
                        KT[prow, hp, jsl],
                        QT[prow, hp, mb * QB + cq : (mb + 1) * QB],
                        tile_position=(64 * s, 0),
                    )
                pt = ptpool.tile([P, 1024], bf16, tag="pt", name="pt")
                if t < 0:
                    nc.scalar.activation(pt[:], ps[:], Exp)
                else:
                    c0 = t * P
                    ps3 = ps.rearrange("p (s q) -> p s q", s=2)
                    pt3 = pt.rearrange("p (s q) -> p s q", s=2)
                    # cols [0, c0) are never read (AV starts at cq=c0)
                    nc.scalar.activation(pt3[:, :, c0:], ps3[:, :, c0:], Exp)
                    # causal: keep iff (q - j) >= 0
                    for s in range(2):
                        nc.vector.tensor_mul(
                            pt3[:, s, c0:],
                            pt3[:, s, c0:],
                            M0[:, : QB - c0],
                        )
                return pt

            def qk_exp(mb, jc):
                return [qk_exp_hp(mb, jc, 0), qk_exp_hp(mb, jc, 1)]

            # ---- fused attention + deferred proj/norm/outproj loop ----
            # PRE chunks of the next query-block's QK+exp are pre-issued at
            # the end of each block's loop: their exps fill the ScalarE idle
            # gaps at block transitions, shortening the exp-paced late phase
            PRE = 4
            pre = {}

            # prologue: fi=0 projections, then mb0's four chunk QKs for hp0
            # (they only need fi=0), then fi=1 and the hp1 halves — the
            # first exp fires right after two projection groups instead of
            # waiting for all four
            qk_group(0, 0, "q")()
            qk_group(0, 0, "k")()
            for jc in range(4):
                pre.setdefault((0, jc), []).append(qk_exp_hp(0, jc, 0))
            qk_group(0, 1, "q")()
            qk_group(0, 1, "k")()
            for jc in range(4):
                pre[(0, jc)].append(qk_exp_hp(0, jc, 1))

            prev_po4 = None
            for mb in range(NB):
                njc = 4 * (mb + 1)
                # deferred work, spread across this mb's chunk boundaries:
                # norm of mb-1 early (starts the collective asap); agb loads
                # and outproj of mb-2 (that AllGather finished long ago, so
                # the loads never head-of-line-block the sync DMA queue);
                # projections for query-block mb+1 fill the remaining slots.
                # slot layout: norm of mb-1 at slot 1 (trigger the collective
                # asap); agb loads of mb-2 at slot 2 (their AllGather is old
                # enough to not stall the sync queue for long); projections
                # for mb+1 spread over the middle; outproj of mb-2 at the END
                # (PE is in-order — its MMs must not enqueue before the
                # AllGather result is certain to have landed).
                slots = {}
                if prev_po4 is not None:
                    slots.setdefault(1, []).append(norm_ag(mb - 1, prev_po4))
                if mb >= 2:
                    slots.setdefault(2, []).append(agb_load(mb - 2))
                # (outproj(0..2) are saved for the epilogue: their Vector
                # ot-copies must not precede the final norm chain in the
                # Vector FIFO, and they double as AllGather filler)
                #
                # Projection schedule: Q/K projections front-loaded (they
                # gate the exp stream, which paces the whole second half);
                # V projections trail (only AVs consume them) — each mb's
                # V groups are emitted inside that mb's own loop, where the
                # PE has slack because the loop is exp-paced.
                # per-hp pre-issue: phase A (hp0) right after the fi=0
                # projections, phase B (hp1) after fi=1 — the hp0 exps
                # start ~4us earlier at each block transition
                def pre_issue_hp(nxt, n, hp):
                    def emit():
                        nxt_order = list(range(4 * nxt, 4 * nxt + 4)) + list(
                            range(0, 4 * nxt)
                        )
                        for jc2 in nxt_order[:n]:
                            pre.setdefault((nxt, jc2), []).append(
                                qk_exp_hp(nxt, jc2, hp)
                            )

                    return emit

                post = []
                if mb == 0:
                    # emitted AFTER mb0's four chunk QKs (not interleaved):
                    # the four exps then run back-to-back while the PE does
                    # these projections, instead of dribbling between them
                    post = [
                        qk_group(1, 0, "q"),
                        qk_group(1, 0, "k"),
                        pre_issue_hp(1, 6, 0),
                        qk_group(1, 1, "q"),
                        qk_group(1, 1, "k"),
                        pre_issue_hp(1, 6, 1),
                        v_group(0), v_group(1), v_group(2), v_group(3),
                    ]
                elif mb == 1:
                    lay = {
                        0: [qk_group(2, 0, "q"), v_group(4)],
                        1: [qk_group(2, 0, "k"), v_group(5)],
                        2: [pre_issue_hp(2, 6, 0), v_group(6)],
                        3: [qk_group(2, 1, "q"), v_group(7)],
                        4: [qk_group(2, 1, "k")],
                        # half of mb3's Q/K projections ride here, half in
                        # mb2: balances PE load across both exp-fed regions
                        5: [pre_issue_hp(2, 6, 1), qk_group(3, 0, "q")],
                        6: [qk_group(3, 0, "k")],
                    }
                    for i, es in lay.items():
                        slots.setdefault(i, []).extend(es)
                elif mb == 2:
                    slots.setdefault(0, []).append(qk_group(3, 1, "q"))
                    slots.setdefault(1, []).append(qk_group(3, 1, "k"))
                    for i in range(4):
                        slots.setdefault(1 + i, []).append(v_group(8 + i))
                else:
                    # own V groups: v(4mb+i) at slot 1+i, consumed by the
                    # diag AVs starting at idx LOOKAHEAD
                    for i in range(4):
                        slots.setdefault(1 + i, []).append(v_group(12 + i))

                # one [65, 4, 512] packed accumulator for all 4 heads
                po4 = psO.tile([DIM_HEAD + 1, GROUPS, QB], f32, name="po4")

                LOOKAHEAD = 4
                pts = {}
                # diagonal (narrow) chunks first (see baseline notes)
                order = list(range(4 * mb, njc)) + list(range(0, 4 * mb))

                def emit_av(jc, mb=mb, po4=po4, order=order, pts=pts):
                    cq = max(0, jc - 4 * mb) * P
                    for hp in range(2):
                        for s in range(2):
                            head = 2 * hp + s
                            nc.tensor.matmul(
                                po4[:, head, cq:],
                                V_sb[:, jc, head, :],
                                pts[jc][hp][:, s * QB + cq : (s + 1) * QB],
                                start=(jc == order[0]),
                                stop=(jc == order[-1]),
                                skip_group_check=True,
                            )
                    del pts[jc]

                # pre-issue the next block's diag chunks (QK+exp only; AVs
                # run in that block's own loop). Emitted as early as the
                # next block's Q/K projections allow, so the exps extend
                # the saturated ScalarE stream through block transitions.
                def pre_issue(nxt, n=PRE):
                    def emit():
                        nxt_order = list(range(4 * nxt, 4 * nxt + 4)) + list(
                            range(0, 4 * nxt)
                        )
                        for jc2 in nxt_order[:n]:
                            pre[(nxt, jc2)] = qk_exp(nxt, jc2)

                    return emit

                if mb == 2:
                    slots.setdefault(6, []).append(pre_issue(3, 6))

                for idx, jc in enumerate(order):
                    for e in slots.get(idx, ()):
                        e()
                    if (mb, jc) in pre:
                        pts[jc] = pre.pop((mb, jc))
                    else:
                        pts[jc] = qk_exp(mb, jc)
                    if idx >= LOOKAHEAD:
                        emit_av(order[idx - LOOKAHEAD])
                for e in post:
                    e()
                for idx2 in range(max(0, njc - LOOKAHEAD), njc):
                    emit_av(order[idx2])
                prev_po4 = po4

            # epilogue: agb of mb=2 BEFORE the last collective is enqueued;
            # mb=0 and mb=2 out-projs (deliberately held back) then fill the
            # PE under the final AllGather; the last agb loads go on the
            # Scalar engine's DMA queue so their AG3-done gate never blocks
            # the sync queue's outT stores.
            # (instructions emitted after a collective_compute are gated on
            # its completion — so all overlappable work must precede the
            # final AllGather's ENQUEUE. The norm compute chain is emitted
            # first so its Vector/Scalar ops aren't queued behind the
            # fillers'; the collective trigger itself is emitted after the
            # fillers — it only data-waits on ag_in, so it still fires as
            # soon as the chain lands, while the fillers escape its gate.)
            agb_load(NB - 2)()
            # one out-proj group first: fills the PE while the final norm's
            # Ln/Exp run on ScalarE (its ot copy beats po_sb in the Vector
            # FIFO but only waits on these first 8 MMs — no entanglement)
            outproj(1, 0)()
            ag3_trigger = norm_ag(NB - 1, prev_po4, defer_cc=True, fast=True)()
            outproj(1, 1)()
            outproj(0, 0)()
            outproj(0, 1)()
            outproj(NB - 2, 0)()
            outproj(NB - 2, 1)()
            # paced keep-warm chain through the final AllGather window: the
            # PE re-throttles to 1.2GHz after a >3.4us idle gap, making the
            # last out-proj ~2x slower. Dummy MMs paced by ScalarE copies
            # (ScalarE is otherwise idle here; the Vector queue — which
            # carries the critical ot copies — is deliberately untouched).
            # 8 iterations only: the outproj fillers above already span the
            # first ~12us of the AllGather window; a longer chain blocks
            # outproj(3) behind it in the in-order PE FIFO when the
            # collective lands early (measured: kw=22 cost ~10us)
            for i in range(8):
                kwp = psS.tile([P, 1024], f32, tag="S", name="kw")[:, :QB]
                nc.tensor.matmul(kwp, wo_bf[:, 0, 0:P], agbs[NB - 2][:, 0, :])
                kws = work.tile([P, QB], f32, tag="kws", bufs=1)
                nc.scalar.copy(kws[:], kwp)
            ag3_trigger()
            # final agb spread over three queues so the 1MiB lands ~2us
            # after the AllGather completes; scalar gets a late sub-DMA
            # since its queue drains the keep-warm copies first
            agb_load(NB - 1, queues=[nc.gpsimd, nc.sync, nc.scalar])()
            outproj(NB - 1, 0)()
            outproj(NB - 1, 1)()

    nc.finalize()
    return nc


def _get_nc():
    global _cached
    if _cached is None:
        _cached = _build_nc()
    return _cached


def _m0_const():
    import ml_dtypes

    m = (np.arange(QB)[None, :] >= np.arange(P)[:, None]).astype(np.float32)
    return m.astype(ml_dtypes.bfloat16)


def kernel(x, mask, Wq, Wk, Wv, Wo):
    import ml_dtypes

    bf = ml_dtypes.bfloat16
    x = np.asarray(x, dtype=np.float32)
    mask = np.asarray(mask)
    # this problem's padding mask is all-True (spec fill: ones); the kernel
    # relies on that (only the causal mask is applied on device).
    assert mask.all(), "kernel specialized for all-ones padding mask"
    Wq = np.asarray(Wq, dtype=np.float32).astype(bf)
    Wk = np.asarray(Wk, dtype=np.float32).astype(bf)
    Wv = np.asarray(Wv, dtype=np.float32).astype(bf)
    Wo = np.asarray(Wo, dtype=np.float32).astype(bf)

    from concourse import bass_utils

    nc = _get_nc()

    xTs = [np.ascontiguousarray(x[b].T.astype(bf)) for b in range(B)]
    m0 = _m0_const()
    # the device stores each block's attn output in (d, h) row order (DMA
    # contiguity); permute wo's rows within each head-group block to match
    Wo_p = np.ascontiguousarray(
        Wo.reshape(GROUPS, GROUPS, DIM_HEAD, DIM).transpose(0, 2, 1, 3).reshape(DIM, DIM)
    )
    in_maps = []
    for c in range(NCORES):
        b, g = divmod(c, GROUPS)
        gsl = slice(g * GDIM, (g + 1) * GDIM)
        in_maps.append(
            {
                "xT": xTs[b],
                "wq": np.ascontiguousarray(Wq[:, gsl]),
                "wk": np.ascontiguousarray(Wk[:, gsl]),
                "wv": np.ascontiguousarray(Wv[:, gsl]),
                "wo": np.ascontiguousarray(Wo_p[:, gsl]),
                "m0": m0,
            }
        )

    res = bass_utils.run_bass_kernel_spmd(nc, in_maps, core_ids=list(range(NCORES)))

    out = np.empty((B, N, DIM), dtype=np.float32)
    for c in range(NCORES):
        b, g = divmod(c, GROUPS)
        out[b, :, g * GDIM : (g + 1) * GDIM] = (
            res.results[c]["outT"].astype(np.float32).T
        )
    return out

